# revision 17
# baseline (speedup 1.0000x reference)
"""Trainium2 Bass kernel for nn_AggregateGCN (3-layer GCN, batched graph,
agent-node readout).

Math (reference): deg-normalized GraphConv x2 on top of a linear+relu input
projection, then a final projection of the 64 agent rows (nodes 0, N, 2N, ...).
Only the 64 agent rows of the last conv are read, so the exact computation
is the backward dependency cone:
  layer2 needs edges into the 64 agents (~2k edges -> ~2k distinct sources S1)
  layer1 needs edges into S1 (~64k edges), with per-edge h0 = relu(x@w_lin+b)
Degrees (in/out over ALL 4M edges) feed the symmetric normalization; the
host folds ALL norm scales into the layer-2 adjacency weights (a2t) so the
device never applies a scale (relu commutes with positive per-slot scales).

Sharding: agents are LPT-assigned to cores (8 each, balancing cone edge
counts) with each core's full cone replicated -> zero cross-device traffic;
the host scatters the per-core [8, 64] outputs back to global row order.

v3 design notes (HW-measured findings):
  - NO DoubleRow: DR disables Fast Weight Load; at SpMM width (FD=88) the
    ldweights overhead dominated (~180ns/MM, +12us). Plain fp8 matmuls
    stream 1 col/cycle with FWL-hidden weight loads (~107ns per 256-col MM).
  - DR matmuls at different tile_positions sharing a PSUM bank also crash
    the device (NRT_EXEC_UNIT_UNRECOVERABLE) - all avoided now.
  - binding resources are the PE stream (~11.7us) and the PSUM->SBUF relu
    eviction path (ACT+DVE only engines with PSUM ports, ~11us):
      * evictions grouped in 4-chunk tiles ([128, 1024] fp32, 2 banks per
        instruction, 3-buf rotation so ACT/DVE never wait on the PE)
      * S1 slots live in 6 groups of <=64 (dmax~48) instead of 3 halves of
        128: halves the SpMM sel stream (PE) and the sel DMA bytes
      * both aggT hid-halves accumulate in ONE PSUM bank: leading zero
        matmuls (rhs = zero sel block) write explicit zeros so real chunks
        accumulate with start=False (a start=True would re-mark the whole
        2KB zero-region and drop earlier accumulations)
  - xe/wlin/hs0/sel all fp8e4 (DMA 22.7KB -> ~14KB/partition); power-of-2
    scales (xe*4, wlin*8) dodge fp8 denormals; the exact 1/32 and the
    io1/in2 norms fold into fp16 a2t on host. h1/layer-2 stay fp16.
  - PSUM budget: 6 (h0 3 bufs x 2 banks) + 1 (aggT) + 1 (mlp) = 8 banks.
  - ONE packed constants param; steady-state bodies prefetch a whole body
    ahead; stage B of body i runs injected early inside body i+1.
Non-zero-bias inputs fall back to an exact numpy host path (the reference
generator always uses zero biases).
"""
import os
import sys

sys.path.insert(0, "/opt/trn_rl_repo")

PROBE = set(os.environ.get("KPROBE", "").split(","))  # timing ablations

import numpy as np
import concourse.bass as bass
import concourse.bacc as bacc
import concourse.mybir as mybir
import concourse.tile as tile

F32 = mybir.dt.float32
F16 = mybir.dt.float16
F8 = mybir.dt.float8e4
AF = mybir.ActivationFunctionType
ALU = mybir.AluOpType
U8 = mybir.dt.uint8

# problem constants (fixed by the spec)
B = 64          # graphs
NPG = 2048      # nodes per graph
TOTAL = B * NPG
IN_DIM = 128
HID = 256
EMB = 64
NCORES = 8
AGENTS_PER_CORE = B // NCORES      # 8
NGRP = 6                           # S1 slot groups (pairs feed 128-row h1)
GSLOT = 64                         # loose slot capacity per group (host)
P = 128

SX = 4.0                           # host scale on xe   (power of 2)
SW = 8.0                           # host scale on wlin (power of 2)
EV_G = 4                           # chunks per eviction tile (2 PSUM banks)
EV_BUFS = 3                        # h0 PSUM tiles in flight
N_WARM = 7                         # fp32 warm-up matmuls (PE clock ramp)

# packed constants param layout (byte columns; mixed dtypes, uint8 carrier)
CB_WLIN = 0                                   # [128, 256] fp8 (x SW)


def cb_layout(dmax):
    zsel = CB_WLIN + HID                      # dmax zero fp8 bytes
    a2t = zsel + dmax                         # [128, 3, 8] fp16 (norms folded)
    a2t += a2t % 2
    wc0 = a2t + NGRP // 2 * AGENTS_PER_CORE * 2   # [128, 2, 256] fp16
    wc1 = wc0 + 2 * HID * 2                   # [128, 2, 256] fp16
    wemb = wc1 + 2 * HID * 2                  # [128, 2, 64] fp16
    end = wemb + 2 * EMB * 2
    return zsel, a2t, wc0, wc1, wemb, end


def slice_plan(nchunk, cold=False):
    """Chunk-group sizes for the xs slice DMAs. Steady state prefetches a
    whole body ahead -> ONE transfer; cold start streams in slices so
    compute can begin before the full stream arrives."""
    if not cold:
        return [nchunk]
    plan = [EV_G]
    while sum(plan) < nchunk:
        plan.append(min(3 * EV_G, nchunk - sum(plan)))
    return plan


def build_program_zb(nch: int, dmax: int, repeat: int = 1) -> bass.Bass:
    """nch = chunks per slot group; dmax = used slots per group (pad 8)."""
    nchunk = NGRP * nch
    cwb = P + dmax                     # bytes per chunk per partition in xs
    AG = AGENTS_PER_CORE
    CB_BYTES = cb_layout(dmax)[-1]

    nc = bacc.Bacc(
        "TRN2", target_bir_lowering=False, debug=False, num_devices=NCORES
    )
    xs = nc.declare_dram_parameter("xs", [P, nchunk * cwb], U8, isOutput=False)
    cbf = nc.declare_dram_parameter("cbf", [P, CB_BYTES], U8, isOutput=False)
    out = nc.declare_dram_parameter("out", [AG, EMB], F32, isOutput=True)

    with tile.TileContext(nc) as tc:
        with (
            tc.tile_pool(name="const", bufs=2) as cp,
            tc.tile_pool(name="hs0p", bufs=EV_BUFS + 2) as hs0p,
            tc.tile_pool(name="copies", bufs=6) as cop,
            tc.tile_pool(name="stage", bufs=2) as stp,
            tc.tile_pool(name="h0ps", bufs=EV_BUFS, space="PSUM") as h0psp,
            tc.tile_pool(name="aggtps", bufs=1, space="PSUM") as aggtpsp,
            tc.tile_pool(name="mlpps", bufs=1, space="PSUM") as mlppsp,
            # PSUM banks: h0 (3 bufs x 2 banks) + aggT (1) + mlp/warm (1) = 8
        ):
            # PE warm-up ONCE, outside the repeat loop
            wu_t = cp.tile([P, P], F32, tag="wu")
            nc.gpsimd.memset(wu_t[:], 0.25)
            warm_ps = mlppsp.tile([P, 2, HID], F32, tag="mlp", name="warm")
            for _w in range(N_WARM):
                nc.tensor.matmul(
                    out=warm_ps[:, 0, :64], lhsT=wu_t[:], rhs=wu_t[:, :64],
                    start=True, stop=True,
                )

            nsets = [0]

            def alloc_set():
                i = nsets[0]
                nsets[0] += 1
                return dict(
                    cbf=cp.tile([P, CB_BYTES], U8, tag="cbf", name=f"cbf{i}"),
                    xs=cp.tile([P, nchunk * cwb], U8, tag="xs",
                               name=f"xs{i}"),
                )

            def _dma(ts, cold=False):
                emit_zb_dma(nc, ts, nchunk, cwb, xs, cbf, cold)

            def _compute(ts, inject=None):
                with nc.allow_low_precision(
                        reason="fp8 intermediates; ~7e-3 vs 2e-2 gate"):
                    return emit_zb_compute(nc, hs0p, cop, stp, h0psp,
                                           aggtpsp, mlppsp, ts, nch,
                                           dmax, out, inject=inject)

            # Software-pipelined repeat loop: two tile sets A/B prefetched
            # before the loop; each body computes from a set then refills it
            # for the body after next. Stage B of body i runs injected early
            # inside body i+1.
            UNROLL = 24
            if repeat == 1:
                A = alloc_set()
                _dma(A, cold=True)
                _compute(A)()
            elif repeat == 2:
                A, Bs = alloc_set(), alloc_set()
                _dma(A, cold=True)
                _dma(Bs)
                sb = _compute(A)
                _compute(Bs, inject=sb)()
            else:
                sets = [alloc_set(), alloc_set()]
                _dma(sets[0], cold=True)
                _dma(sets[1])
                pend = [None]

                def _tick(i):
                    prev = pend[0]
                    S = sets[i % 2]
                    Sprev = sets[(i + 1) % 2]

                    def _inject():
                        if prev is not None:
                            prev()
                            _dma(Sprev)

                    pend[0] = _compute(S, inject=_inject)

                def _flush(i):
                    if pend[0] is not None:
                        pend[0]()
                        _dma(sets[i % 2])
                        pend[0] = None

                if repeat // UNROLL > 0:
                    with tc.For_i(0, repeat // UNROLL, 1):
                        for i in range(UNROLL):
                            _tick(i)
                        _flush(UNROLL - 1)
                for i in range(repeat % UNROLL):
                    _tick(i)
                _flush(repeat % UNROLL - 1)
    nc.compile()
    return nc


def emit_zb_dma(nc, ts, nchunk, cwb, xs, cbf, cold=False):
    if "nodma" in PROBE:
        return
    nc.sync.dma_start(out=ts["cbf"][:], in_=cbf[:])
    xs_t = ts["xs"]
    c0 = 0
    for n in slice_plan(nchunk, cold):
        nc.sync.dma_start(out=xs_t[:, c0 * cwb:(c0 + n) * cwb],
                          in_=xs[:, c0 * cwb:(c0 + n) * cwb])
        c0 += n


def emit_zb_compute(nc, hs0p, cop, stp, h0psp, aggtpsp, mlppsp,
                    ts, nch, dmax, out, inject=None):
    AG = AGENTS_PER_CORE
    cwb = P + dmax
    nchunk = NGRP * nch
    ntiles = -(-nchunk // EV_G)
    NPAIR = NGRP // 2                  # slot-group pairs = h1/stage rows of 3
    CB_ZSEL, CB_A2T, CB_WC0, CB_WC1, CB_WEMB, _ = cb_layout(dmax)

    cbf_t = ts["cbf"]
    xs_t = ts["xs"]
    wlin_t = cbf_t[:, CB_WLIN:CB_ZSEL].bitcast(F8)
    zsel = cbf_t[:, CB_ZSEL:CB_ZSEL + dmax].bitcast(F8)
    a2t_t = cbf_t[:, CB_A2T:CB_WC0].bitcast(F16).rearrange(
        "p (c n) -> p c n", n=AG)
    wc0_t = cbf_t[:, CB_WC0:CB_WC1].bitcast(F16).rearrange(
        "p (c n) -> p c n", n=HID)
    wc1_t = cbf_t[:, CB_WC1:CB_WEMB].bitcast(F16).rearrange(
        "p (c n) -> p c n", n=HID)
    wemb_t = cbf_t[:, CB_WEMB:].bitcast(F16).rearrange(
        "p (c n) -> p c n", n=EMB)

    def xe_ap(c):
        return xs_t[:, c * cwb:c * cwb + P].bitcast(F8)

    def sel_ap(c):
        return xs_t[:, c * cwb + P:(c + 1) * cwb].bitcast(F8)

    hs1_t = stp.tile([P, NPAIR, HID], F16, tag="hs1")
    aggT_ps = [None]
    pair_sb = [None] * NPAIR
    h1_ps = [None]

    def emit_h0_tile(t):
        """h0 matmuls for tile t (EV_G chunks) + one grouped relu evict."""
        g = min(EV_G, nchunk - t * EV_G)
        h0_ps = h0psp.tile([P, EV_G * HID], F32, tag="h0")
        for j in range(g if "noh0" not in PROBE else 0):
            nc.tensor.matmul(
                out=h0_ps[:, j * HID:(j + 1) * HID],
                lhsT=xe_ap(t * EV_G + j), rhs=wlin_t,
                start=True, stop=True,
            )
        hs0_t = hs0p.tile([P, EV_G * HID], F8, tag="hs0")
        if "noevict" in PROBE:
            return hs0_t
        w = g * HID
        if t % 2 == 0:
            nc.scalar.activation(hs0_t[:, :w], h0_ps[:, :w], AF.Relu)
        else:
            nc.vector.tensor_scalar(
                out=hs0_t[:, :w], in0=h0_ps[:, :w], scalar1=0.0,
                scalar2=None, op0=ALU.max)
        return hs0_t

    h1_due = []   # pair index whose h1 should be emitted before next tile

    def emit_h1(pr):
        if "noh1" in PROBE:
            return
        # h1 pair pr: [2*dmax slots, 2, 256] into the SHARED mlp-bank tile
        # (pr even -> fresh tile + slot 0, pr odd -> slot 1 of same tile)
        if pr % 2 == 0:
            h1_ps[0] = mlppsp.tile([P, 2, HID], F32, tag="mlp",
                                   name=f"h1_{pr}")
        sl = pr % 2
        for k in range(HID // P):
            nc.tensor.matmul(
                out=h1_ps[0][:2 * dmax, sl, :],
                lhsT=pair_sb[pr][:, k, :, :], rhs=wc0_t[:, k, :],
                start=(k == 0), stop=(k == HID // P - 1),
                skip_group_check=True,
            )
        # evict when the tile is full (pairs 0,1 together; pair 2 alone)
        if pr == 1:
            nc.scalar.activation(
                hs1_t[:2 * dmax, 0:2, :].rearrange("p a b -> p (a b)"),
                h1_ps[0][:2 * dmax, :, :].rearrange("p a b -> p (a b)"),
                AF.Relu)
        elif pr == 2:
            nc.vector.tensor_scalar(
                out=hs1_t[:2 * dmax, 2, :], in0=h1_ps[0][:2 * dmax, 0, :],
                scalar1=0.0, scalar2=None, op0=ALU.max)

    def emit_spmm_tile(t, hs0_t):
        """SpMM for tile t's chunks; zero-matmuls open each group's
        shared-bank accumulator, a copy into the pair tile closes it."""
        g = min(EV_G, nchunk - t * EV_G)
        for pr in h1_due:
            emit_h1(pr)
        del h1_due[:]
        if "nospmm" in PROBE:
            return
        hs0_3d = hs0_t[:].rearrange("p (c n) -> p c n", n=HID)
        for j in range(g):
            c = t * EV_G + j
            gr = c // nch
            ci = c % nch
            if ci == 0:
                aggT_ps[0] = aggtpsp.tile([P, 2, dmax], F32, tag="aggT",
                                          name=f"aggT_{gr}")
                for fh in range(2):
                    nc.tensor.matmul(
                        out=aggT_ps[0][:, fh, :],
                        lhsT=hs0_3d[:, j, fh * P:(fh + 1) * P],
                        rhs=zsel, start=True, stop=False,
                        skip_group_check=True,
                    )
            for fh in range(2):
                nc.tensor.matmul(
                    out=aggT_ps[0][:, fh, :],
                    lhsT=hs0_3d[:, j, fh * P:(fh + 1) * P],
                    rhs=sel_ap(c), start=False, stop=(ci == nch - 1),
                    skip_group_check=True,
                )
            if ci == nch - 1:
                pr, sl = gr // 2, gr % 2
                if sl == 0:
                    pair_sb[pr] = cop.tile([P, 2, 2, dmax], F16, tag="daT",
                                           name=f"aT{pr}")
                dst = pair_sb[pr][:, :, sl, :]
                if gr % 2 == 0:
                    nc.scalar.activation(dst, aggT_ps[0][:], AF.Copy)
                else:
                    nc.vector.tensor_copy(out=dst, in_=aggT_ps[0][:])
                if sl == 1:
                    h1_due.append(pr)

    # ---- stage A: pipelined tiles; SpMM trails eviction by one tile ----
    pend_tile = []
    for t in range(ntiles):
        pend_tile.append((t, emit_h0_tile(t)))
        if t == 2 and inject is not None:
            inject()
        if t >= 1:
            emit_spmm_tile(*pend_tile.pop(0))
    while pend_tile:
        emit_spmm_tile(*pend_tile.pop(0))
    for pr in h1_due:
        emit_h1(pr)
    del h1_due[:]

    def stage_b():
        if "nostageb" in PROBE:
            return
        # ---- stage B: layer 2 on the 8 agent rows (fp16, norms in a2t) ----
        h2rT_t = [None, None]
        for oh in range(2):
            a2T_ps = mlppsp.tile([P, 2, HID], F32, tag="mlp",
                                 name=f"a2T{oh}")
            for h in range(NPAIR):
                nc.tensor.matmul(
                    out=a2T_ps[:, 0, :AG],
                    lhsT=hs1_t[:2 * dmax, h, oh * P:(oh + 1) * P],
                    rhs=a2t_t[:2 * dmax, h, :],
                    start=(h == 0), stop=(h == NPAIR - 1),
                    skip_group_check=True,
                )
            a2T_sb = cop.tile([P, AG], F16, tag="da2T", name=f"a2Tsb{oh}")
            nc.vector.tensor_copy(out=a2T_sb[:], in_=a2T_ps[:, 0, :AG])
            h2rT_t[oh] = a2T_sb
        z2T_sb = [None, None]
        for oh in range(2):
            z2_ps = mlppsp.tile([P, 2, HID], F32, tag="mlp",
                                name=f"z2T{oh}")
            for kc in range(2):
                nc.tensor.matmul(
                    out=z2_ps[:, 0, :AG],
                    lhsT=wc1_t[:, kc, oh * P:(oh + 1) * P],
                    rhs=h2rT_t[kc][:],
                    start=(kc == 0), stop=(kc == 1),
                    skip_group_check=True,
                )
            zr_t = cop.tile([P, AG], F16, tag="z2r", name=f"z2r{oh}")
            nc.scalar.activation(zr_t[:], z2_ps[:, 0, :AG], AF.Relu)
            z2T_sb[oh] = zr_t
        out_ps = mlppsp.tile([AG, 2, HID], F32, tag="mlp", name="outps")
        for oh in range(2):
            nc.tensor.matmul(
                out=out_ps[:, 0, :EMB], lhsT=z2T_sb[oh][:],
                rhs=wemb_t[:, oh, :],
                start=(oh == 0), stop=(oh == 1),
                skip_group_check=True,
            )
        out_t = stp.tile([AG, EMB], F32, tag="outt")
        nc.vector.tensor_copy(out=out_t[:], in_=out_ps[:, 0, :EMB])
        nc.sync.dma_start(out=out[:], in_=out_t[:])

    return stage_b


# ---------------------------------------------------------------------------
# host-side preprocessing / packing
# ---------------------------------------------------------------------------

def prepare_inputs(x, src, dst):
    """Host-side integer index preprocessing + sharding. Agents are
    LPT-assigned to cores (8 each, balancing cone edge counts); S1 nodes are
    LPT-assigned to the 6 dst slot-groups by in-degree with a fill cap so
    DMAX (max used slots per group) stays small."""
    deg_out = np.bincount(src, minlength=TOTAL).astype(np.float32)
    deg_in = np.bincount(dst, minlength=TOTAL).astype(np.float32)

    g = dst // NPG                     # graph id of each edge's dst
    is_agent = (dst % NPG) == 0
    g2 = g[is_agent]
    s2_all = src[is_agent]

    loads = np.zeros(B, np.int64)
    for a in range(B):
        loads[a] = deg_in[np.unique(s2_all[g2 == a])].sum()
    bins = [[] for _ in range(NCORES)]
    bl = np.zeros(NCORES, np.int64)
    for a in np.argsort(-loads):
        cands = [i for i in range(NCORES) if len(bins[i]) < AGENTS_PER_CORE]
        i = min(cands, key=lambda i: bl[i])
        bins[i].append(int(a))
        bl[i] += loads[a]

    cores = []
    agent_rows = []                     # global output row per concat position
    nch_needed = 1
    dmax_needed = 1
    for c in range(NCORES):
        agents_g = bins[c]              # graph ids owned by this core
        agent_rows.extend(agents_g)
        am = np.zeros(B, bool)
        am[agents_g] = True
        m2 = is_agent & am[g]
        e2_src = src[m2]
        gl = np.full(B, -1, np.int64)
        gl[agents_g] = np.arange(AGENTS_PER_CORE)
        e2_ag = gl[g[m2]]
        s1 = np.unique(e2_src)
        m1c = s1.size
        assert m1c <= NGRP * GSLOT, f"S1 overflow: {m1c}"
        cap = min(GSLOT, -(-m1c // NGRP) + 2)
        hload = np.zeros(NGRP, np.int64)
        hfill = np.zeros(NGRP, np.int64)
        slot = np.empty(m1c, np.int64)   # loose slot id = grp*GSLOT + fill
        d1 = deg_in[s1].astype(np.int64)
        for i in np.argsort(-d1):
            cands = [hh for hh in range(NGRP) if hfill[hh] < cap]
            hh = min(cands, key=lambda hh: hload[hh])
            slot[i] = hh * GSLOT + hfill[hh]
            hfill[hh] += 1
            hload[hh] += d1[i]
        dmax_needed = max(dmax_needed, int(hfill.max()))
        loc = np.full(TOTAL, -1, dtype=np.int64)
        loc[s1] = slot
        # layer-2 adjacency counts on LOOSE slots (repacked once dmax known)
        a2t = np.zeros((NGRP * GSLOT, AGENTS_PER_CORE), dtype=np.float32)
        np.add.at(a2t, (loc[e2_src], e2_ag), 1.0)
        io1 = np.zeros(NGRP * GSLOT, np.float32)
        io1[loc[s1]] = (np.maximum(deg_in[s1], 1.0)
                        * np.maximum(deg_out[s1], 1.0)) ** -0.5
        agents = np.asarray(agents_g, np.int64) * NPG
        in2 = (np.maximum(deg_in[agents], 1.0) ** -0.5)   # [AG]

        dl = loc[dst]
        es = dl >= 0
        e1_src = src[es]
        e1_slot = dl[es]
        grps = []
        for h in range(NGRP):
            hm = (e1_slot // GSLOT) == h
            grps.append((e1_src[hm], e1_slot[hm] - h * GSLOT))
            nch_needed = max(nch_needed, -(-grps[h][0].size // P))
        cores.append(dict(a2t=a2t, io1=io1, in2=in2, grps=grps))
    return cores, deg_out, nch_needed, dmax_needed, np.asarray(
        agent_rows, np.int64)


def pack_core_zb(core, x, deg_out, nch, dmax, wlin8):
    """Pack one core's interleaved uint8 stream (per chunk: xe fp8
    [feature, edge] | sel fp8 [edge, slot]), plus the packed constants."""
    nchunk = NGRP * nch
    cwb = P + dmax
    f8 = mybir.dt.np(F8)
    CB_ZSEL, CB_A2T, CB_WC0, CB_WC1, CB_WEMB, CB_BYTES = cb_layout(dmax)
    xs3 = np.zeros((P, nchunk, cwb), dtype=np.uint8)
    one8 = np.ones((), dtype=f8)
    for h, (hsrc, hslot) in enumerate(core["grps"]):
        k = hsrc.size
        se = (np.maximum(deg_out[hsrc], 1.0) ** -0.5).astype(np.float32)
        xeh = np.zeros((nch * P, IN_DIM), np.float32)
        xeh[:k] = x[hsrc] * (se * SX)[:, None]
        xe8 = xeh.astype(f8).reshape(nch, P, IN_DIM).transpose(2, 0, 1)
        selh = np.zeros((nch * P, dmax), f8)
        selh[np.arange(k), hslot] = one8
        c0 = h * nch
        xs3[:, c0:c0 + nch, 0:P] = xe8.copy().view(np.uint8)
        xs3[:, c0:c0 + nch, P:] = (
            selh.reshape(nch, P, dmax).transpose(1, 0, 2)
            .copy().view(np.uint8))

    # a2t: loose slots -> compact (grp*dmax + fill), norms folded in, then
    # grouped into NPAIR rows of 2*dmax partitions
    a2l = core["a2t"] * core["io1"][:, None] / (SX * SW)
    a2l = a2l * core["in2"][None, :]
    a2c = a2l.reshape(NGRP, GSLOT, AGENTS_PER_CORE)[:, :dmax, :]
    a2c = a2c.reshape(NGRP // 2, 2 * dmax, AGENTS_PER_CORE)

    cbf = np.zeros((P, CB_BYTES), dtype=np.uint8)
    cbf[:, CB_WLIN:CB_WLIN + HID] = wlin8.view(np.uint8)
    a2p = np.zeros((P, NGRP // 2, AGENTS_PER_CORE), np.float16)
    a2p[:2 * dmax] = a2c.transpose(1, 0, 2).astype(np.float16)
    cbf[:, CB_A2T:CB_WC0] = a2p.reshape(P, -1).copy().view(np.uint8)
    return dict(xs=xs3.reshape(P, nchunk * cwb), cbf=cbf)


def make_in_maps(x, src, dst, w_lin, b_lin, w_c0, b_c0, w_c1, b_c1,
                 w_emb, b_emb):
    """Host preprocessing -> (in_maps, cfg, agent_rows)."""
    x = np.asarray(x, dtype=np.float32)
    src = np.asarray(src).astype(np.int64)
    dst = np.asarray(dst).astype(np.int64)
    cores, deg_out, nch, dmax, agent_rows = prepare_inputs(x, src, dst)
    dmax = min(GSLOT, -(-dmax // 8) * 8)
    f8 = mybir.dt.np(F8)

    def pcn(w, n):
        return (np.asarray(w, np.float16).reshape(HID // P, P, n)
                .transpose(1, 0, 2).reshape(P, (HID // P) * n)
                .copy().view(np.uint8))

    wlin8 = (np.asarray(w_lin, np.float32) * SW).astype(f8)  # [128, 256]

    CB_ZSEL, CB_A2T, CB_WC0, CB_WC1, CB_WEMB, CB_BYTES = cb_layout(dmax)
    wc0p, wc1p, wembp = pcn(w_c0, HID), pcn(w_c1, HID), pcn(w_emb, EMB)
    in_maps = []
    for c in range(NCORES):
        m = pack_core_zb(cores[c], x, deg_out, nch, dmax, wlin8)
        m["cbf"][:, CB_WC0:CB_WC1] = wc0p
        m["cbf"][:, CB_WC1:CB_WEMB] = wc1p
        m["cbf"][:, CB_WEMB:CB_BYTES] = wembp
        in_maps.append(m)
    return in_maps, dict(zero_bias=True, nch=nch, dmax=dmax), agent_rows


def build_program(cfg, repeat: int = 1) -> bass.Bass:
    return build_program_zb(cfg["nch"], cfg["dmax"], repeat=repeat)


def _kernel_numpy(x, src, dst, w_lin, b_lin, w_c0, b_c0, w_c1, b_c1,
                  w_emb, b_emb):
    """Exact host fallback for non-zero biases (never hit by the reference
    generator, which uses zero biases). Segment sums via sort+reduceat."""
    f = np.float64
    n = x.shape[0]
    out_deg = np.bincount(src, minlength=n).astype(f)
    in_deg = np.bincount(dst, minlength=n).astype(f)
    out_norm = np.maximum(out_deg, 1.0) ** -0.5
    in_norm = np.maximum(in_deg, 1.0) ** -0.5
    order = np.argsort(dst, kind="stable")
    sdst = dst[order]
    ssrc = src[order]
    starts = np.flatnonzero(np.r_[True, sdst[1:] != sdst[:-1]])

    def conv(h, W, b):
        hs = (h * out_norm[:, None])[ssrc]
        sums = np.add.reduceat(hs, starts, axis=0)
        agg = np.zeros((n, h.shape[1]), f)
        agg[sdst[starts]] = sums
        return (agg * in_norm[:, None]) @ np.asarray(W, f) + np.asarray(b, f)

    h = np.maximum(np.asarray(x, f) @ np.asarray(w_lin, f)
                   + np.asarray(b_lin, f), 0.0)
    h = np.maximum(conv(h, w_c0, b_c0), 0.0)
    h = np.maximum(conv(h, w_c1, b_c1), 0.0)
    agent = h[np.arange(0, n, NPG)]
    return (agent @ np.asarray(w_emb, f) + np.asarray(b_emb, f)).astype(
        np.float32)


def assemble_out(core_outs, agent_rows):
    """Scatter per-core [8, EMB] outputs back to global agent row order."""
    full = np.empty((B, EMB), np.float32)
    full[agent_rows] = np.concatenate(core_outs, axis=0)
    return full


def kernel(x, src, dst, num_nodes, nodes_per_graph,
           w_lin, b_lin, w_c0, b_c0, w_c1, b_c1, w_emb, b_emb,
           _debug=None) -> np.ndarray:
    from concourse.bass_utils import run_bass_kernel_spmd

    assert int(num_nodes) == TOTAL and int(nodes_per_graph) == NPG
    if (np.any(np.asarray(b_lin)) or np.any(np.asarray(b_c0))
            or np.any(np.asarray(b_c1)) or np.any(np.asarray(b_emb))):
        src = np.asarray(src).astype(np.int64)
        dst = np.asarray(dst).astype(np.int64)
        return _kernel_numpy(np.asarray(x, np.float32), src, dst, w_lin,
                             b_lin, w_c0, b_c0, w_c1, b_c1, w_emb, b_emb)
    in_maps, cfg, agent_rows = make_in_maps(
        x, src, dst, w_lin, b_lin, w_c0, b_c0, w_c1, b_c1, w_emb, b_emb)

    nc = build_program(cfg)
    core_ids = list(range(NCORES))
    if _debug is not None:
        _debug["nc"] = nc
        _debug["in_maps"] = in_maps
        _debug["cfg"] = cfg
    res = run_bass_kernel_spmd(nc, in_maps, core_ids)
    return assemble_out([res.results[c]["out"] for c in range(NCORES)],
                        agent_rows)
